# revision 1
# baseline (speedup 1.0000x reference)
"""nn_Attention_42374147342446 — GNN message-passing attention, 8-way sharded.

Sharding (per spec hint): data-parallel over batch B=4 and receiver half
(i-axis, 512 rows each) -> 8 shards, one per NeuronCore. K/V (senders) are
computed per-shard from the full batch-b token set (replicated within the
batch's 2 shards); edge_features / mask / logits shard cleanly on (b, i-half).

Shard c (c = 0..7):  b = c // 2,  i in [512*(c%2), 512*(c%2) + 512).

kernel() takes FULL unsharded inputs, returns the FULL (4, 1024, 512) output.
Self-contained: shapes hardcoded, no sibling imports.
"""

import numpy as np

B, N, F = 4, 1024, 512
H, D = 8, 64
E = 16
LN_EPS = 1e-5
NSH = 2              # i-halves per batch
SH = N // NSH        # 512 receiver rows per shard
NCORES = B * NSH     # 8


def _shard_fn(x_full, x_q, edge_sl, mask_sl, ln_scale, ln_offset, Wq, Wk, Wv, We, Wo):
    """Compute one shard: full-batch senders (N tokens), SH receivers."""
    import jax
    import jax.numpy as jnp

    def ln(t):
        mu = jnp.mean(t, axis=-1, keepdims=True)
        var = jnp.var(t, axis=-1, keepdims=True)
        return (t - mu) * jax.lax.rsqrt(var + LN_EPS) * ln_scale + ln_offset

    r_full = ln(x_full)                                   # (N, F) senders
    r_q = ln(x_q)                                         # (SH, F) receivers
    q = (r_q @ Wq).reshape(SH, H, D)
    k = (r_full @ Wk).reshape(N, H, D)
    v = (r_full @ Wv).reshape(N, H, D)
    # logits (i, j, h): QK^T + edge bias, softmax over senders j (axis 1)
    # edge_sl arrives fp16 (halves host->device staging of the 256 MB tensor);
    # upcast before the contraction so bias math stays fp32.
    logits = jnp.einsum("ihf,jhf->ijh", q, k) + edge_sl.astype(jnp.float32) @ We
    w = jax.nn.softmax(logits, axis=1)
    w = w * mask_sl[..., None]                            # post-softmax mask
    out = jnp.einsum("ijh,jhv->ihv", w, v)
    out = out.reshape(SH, H * D) * (1.0 / jnp.sqrt(jnp.float32(D)))
    return out @ Wo + x_q                                 # residual


def _stack_shards(receiver_input, edge_features, mask):
    # Shard c = b*NSH + ih <-> (b = c//NSH, ih = c%NSH), so the shard split is
    # a pure reshape view for every tensor sharded on (b, i-half) — no 256 MB
    # host copy of edge_features before staging.
    xq = np.ascontiguousarray(receiver_input).reshape(NCORES, SH, F)
    eg = np.ascontiguousarray(edge_features).reshape(NCORES, SH, N, E)
    eg = eg.astype(np.float16)  # transfer-precision only; upcast on device
    mk = np.ascontiguousarray(mask).reshape(NCORES, SH, N)
    xf = np.repeat(receiver_input, NSH, axis=0)   # senders: full batch-b tokens
    return xf, xq, eg, mk


def _unstack(out_sh):
    out = np.empty((B, N, F), dtype=np.float32)
    for c in range(NCORES):
        b, ih = c // NSH, c % NSH
        out[b, ih * SH:(ih + 1) * SH] = out_sh[c]
    return out


def kernel(receiver_input, edge_features, mask, ln_scale, ln_offset,
           Wq, Wk, Wv, We, Wo):
    receiver_input = np.asarray(receiver_input, dtype=np.float32)
    edge_features = np.asarray(edge_features, dtype=np.float32)
    mask = np.asarray(mask, dtype=np.float32)
    weights = [np.asarray(w, dtype=np.float32)
               for w in (ln_scale, ln_offset, Wq, Wk, Wv, We, Wo)]

    xf, xq, eg, mk = _stack_shards(receiver_input, edge_features, mask)

    import jax

    # Preferred: pmap across the 8 NeuronCores (weights replicated).
    try:
        devs = jax.devices()
        if len(devs) >= NCORES:
            pfn = jax.pmap(
                _shard_fn,
                in_axes=(0, 0, 0, 0) + (None,) * 7,
                devices=devs[:NCORES],
            )
            out_sh = np.asarray(pfn(xf, xq, eg, mk, *weights))
            return _unstack(out_sh.astype(np.float32))
    except Exception as exc:  # pragma: no cover - device-path fallback
        import sys
        print(f"[kernel] pmap path failed ({exc!r}); falling back", file=sys.stderr)

    # Fallback 1: per-device jit, sequential.
    try:
        devs = jax.devices()
        outs = []
        for c in range(NCORES):
            d = devs[c % len(devs)]
            f = jax.jit(_shard_fn, device=d)
            outs.append(np.asarray(f(xf[c], xq[c], eg[c], mk[c], *weights)))
        return _unstack(np.stack(outs).astype(np.float32))
    except Exception as exc:  # pragma: no cover
        import sys
        print(f"[kernel] per-device path failed ({exc!r}); cpu fallback",
              file=sys.stderr)

    # Fallback 2: plain CPU jax (always correct).
    with jax.default_device(jax.devices("cpu")[0]):
        outs = [np.asarray(jax.jit(_shard_fn)(xf[c], xq[c], eg[c], mk[c], *weights))
                for c in range(NCORES)]
    return _unstack(np.stack(outs).astype(np.float32))

